# revision 82
# baseline (speedup 1.0000x reference)
"""Trainium2 Bass/Tile kernel for nn_AttnBlock_29712583753795.

Per sample (B=16, C=512, H=W=64, n=4096):
    xn  = groupnorm(x; 16 groups, w1, b1)
    kv  = kv_w @ xn + kv_b                  (1x1 conv -> [2C, n])
    k, v = split(kv)
    q   = softmax_c(k) * C^-0.5
    k   = softmax_n(k)
    ctx = k @ v.T                           [C, C]
    o2  = ctx.T @ q                         [C, n]
    out = out_w @ gelu(groupnorm(o2; w2, b2)) + out_b
    return xn + out

Sharding: pure data-parallel over batch. 2 samples per NeuronCore, 8 cores.

Design notes (cost model = instruction_cost_v2 TimelineSim):
  * matmul cost = out-free-size cycles (bf16/f32r 1 cyc/row); K<=128 and
    M<=128 are free -> minimize total streamed output columns.
  * all large matmuls in bf16, fp32 PSUM accumulation.
  * k computed ONCE in [n, d] layout; the [d, n] copy for the attention
    contraction comes from PE transposes (128 cyc/tile vs 2048 recompute).
  * R[d] = sum_n e^k via free-dim-1 rider matmuls in 4 separate PSUM banks
    (a 2KB zero region admits one accumulation group).
  * per-d factors fold into the ctx drain: ctx' = (ctx/R + vb)*e^kb with
    e^kb*vb precomputed host-side; S rides as sclq = e^kb*sqrt(C).
  * o2 stays in SBUF (bf16); output DMA'd as bf16 and upcast on host.
  * software pipelining: phase 1 of sample s+1 is split around phase 3 of
    sample s, hiding the GN2 parameter latency and the ACT table loads
    (exp/gelu live in different ACT table sets).
  * elementwise work spread across DVE/ACT/GPSIMD (GPSIMD may not touch
    PSUM) so phase 2 stays PE-bound; GN1 stats of sample s+1 stream one
    chunk per phase-1 nt of sample s.

Cost-model (instruction_cost_v2 TimelineSim) exec: ~362 us/core, PE ~84%
busy (305 us of streamed matmul columns), vs 473 us for the f32r
recompute-k baseline. Verified end-to-end through neuronxcc+PJRT with
rel err ~4e-3 (gate 2e-2).
"""

import sys

for _p in ("/opt/trn_rl_repo",):
    if _p not in sys.path:
        sys.path.insert(0, _p)

import numpy as np

import concourse.bass as bass
import concourse.tile as tile
from concourse import bacc, mybir
from concourse.bass_utils import run_bass_kernel_spmd

F32 = mybir.dt.float32
BF16 = mybir.dt.bfloat16
I32 = mybir.dt.int32
AX = mybir.AxisListType
OP = mybir.AluOpType
AF = mybir.ActivationFunctionType

N_CORES = 8
B, C, H, W = 16, 512, 64, 64
N = H * W                      # 4096 spatial
BPC = B // N_CORES             # 2 samples per core
P = 128                        # partitions
CT = C // P                    # 4 channel tiles
NT = N // P                    # 32 n-tiles (phase 1)
NCH = N // 512                 # 8 n-chunks of 512 (phases 2/3)
GROUPS = 16
GSIZE = C // GROUPS            # 32 channels per group
GN_COUNT = float(GSIZE * N)    # 131072 elements per group
EPS = 1e-5
PEND = 8                       # phase-2 production lookahead
RLAG = 5                       # S-rider lag behind ek2 production
NEWTON = 1                     # rsqrt Newton iterations (err ~4e-6)
K_PRE = 4                      # phase-1 nts emitted before prev phase 3


def build_program(gelu: bool = True, reps: int = 1):
    """Build the per-core Bass program (identical on all 8 cores)."""
    nc = bacc.Bacc("TRN2", target_bir_lowering=False, debug=False,
                   num_devices=N_CORES)

    x_d = nc.dram_tensor("x", [BPC * C, N], BF16, kind="ExternalInput").ap()
    kvw_d = nc.dram_tensor("kvwT", [C, 2 * C], BF16, kind="ExternalInput").ap()
    outw_d = nc.dram_tensor("outwT", [C, C], BF16, kind="ExternalInput").ap()
    prm_d = nc.dram_tensor("prm", [6, CT, P], F32, kind="ExternalInput").ap()
    sclq_d = nc.dram_tensor("sclq", [CT, P], BF16, kind="ExternalInput").ap()
    vbp_d = nc.dram_tensor("vbp", [C, C], BF16, kind="ExternalInput").ap()
    id_d = nc.dram_tensor("ident", [P, P], BF16, kind="ExternalInput").ap()
    gm_d = nc.dram_tensor("gmat", [P, 4], F32, kind="ExternalInput").ap()
    gmT_d = nc.dram_tensor("gmatT", [4, P], F32, kind="ExternalInput").ap()
    out_d = nc.dram_tensor("out", [BPC * C, N], BF16,
                           kind="ExternalOutput").ap()

    gelu_f = AF.Gelu if gelu else AF.Identity

    with tile.TileContext(nc) as tc:
        from contextlib import ExitStack
        with ExitStack() as ctx:
            E = ctx.enter_context
            const = E(tc.tile_pool(name="const", bufs=1))
            xn_pool = E(tc.tile_pool(name="xn", bufs=2 * CT))
            ekt_pool = E(tc.tile_pool(name="ekt", bufs=NT))
            o2_pool = E(tc.tile_pool(name="o2", bufs=NT))
            ctxsb_pool = E(tc.tile_pool(name="ctxsb", bufs=4))
            xap_pool = E(tc.tile_pool(name="xap", bufs=5))
            vt_pool = E(tc.tile_pool(name="vt", bufs=3))
            ek2_pool = E(tc.tile_pool(name="ek2", bufs=PEND + 3))
            g_pool = E(tc.tile_pool(name="g", bufs=8))
            outsb_pool = E(tc.tile_pool(name="outsb", bufs=5))
            bcs_pool = E(tc.tile_pool(name="bcs", bufs=2))
            dump_pool = E(tc.tile_pool(name="dump", bufs=3))
            bdump_pool = E(tc.tile_pool(name="bdump", bufs=1))
            fold_pool = E(tc.tile_pool(name="fold", bufs=3))
            stat_pool = E(tc.tile_pool(name="stat", bufs=4))
            ab_pool = E(tc.tile_pool(name="ab", bufs=6))
            small_pool = E(tc.tile_pool(name="small", bufs=2))

            # PSUM: 8 banks total -> 3 static pools shared across phases.
            quad_ps = E(tc.tile_pool(name="quad_ps", bufs=4, space="PSUM"))
            tri_ps = E(tc.tile_pool(name="tri_ps", bufs=3, space="PSUM"))
            row_ps = E(tc.tile_pool(name="row_ps", bufs=1, space="PSUM"))

            # -------- startup x DMAs first (stats lanes are the startup
            # critical path), staged into xn-pool slots; then constants ----
            seq = [s for _ in range(reps) for s in range(BPC)]
            xst0 = []
            for ct in range(CT):
                rows = slice(seq[0] * C + ct * P, seq[0] * C + (ct + 1) * P)
                xt = xn_pool.tile([P, N], BF16, name="xstage", tag="xnt")
                nc.sync.dma_start(xt, x_d[rows, :])
                xst0.append(xt)

            pcols = []
            for idx in range(6):
                t = const.tile([P, CT], F32, name=f"prm{idx}", tag=f"prm{idx}")
                nc.sync.dma_start(t, prm_d[idx].rearrange("t p -> p t"))
                pcols.append(t)
            w1c, b1c, w2c, b2c, obc, ekbc = pcols
            gm = const.tile([P, 4], F32)
            nc.sync.dma_start(gm, gm_d)
            gmT = const.tile([4, P], F32)
            nc.sync.dma_start(gmT, gmT_d)
            sclq = const.tile([P, CT], BF16)
            nc.sync.dma_start(sclq, sclq_d.rearrange("t p -> p t"))
            ident = const.tile([P, P], BF16)
            nc.sync.dma_start(ident, id_d)
            ones_col = const.tile([P, 1], BF16)
            nc.vector.memset(ones_col, 1.0)
            ones_row = const.tile([1, P], BF16)
            nc.vector.memset(ones_row, 1.0)
            kvw_sb = const.tile([P, CT * 2 * C], BF16)   # [128, 4096]
            outw_sb = const.tile([P, CT * C], BF16)      # [128, 2048]
            vbp_sb = const.tile([P, CT * C], BF16)       # [128, 2048]

            def gn_affine8(st8, wcols, bcols):
                """Batched group-norm affine for 4 ctiles at once.
                st8: [128, 8] (sums in 0:4 by ct, sumsq in 4:8).
                Returns (a_all, b_all) [128, 4]: y = a*x + b."""
                gps8 = tri_ps.tile([4, 8], F32, name="gps8", tag="tri")
                nc.tensor.matmul(gps8, gm, st8, start=True, stop=True)
                gmn8 = stat_pool.tile([4, 8], F32)
                nc.vector.tensor_scalar_mul(gmn8, gps8, 1.0 / GN_COUNT)
                var4 = stat_pool.tile([4, 4], F32)
                nc.vector.tensor_mul(var4, gmn8[:, 0:4], gmn8[:, 0:4])
                nc.vector.tensor_sub(var4, gmn8[:, 4:8], var4)
                nc.vector.tensor_scalar_add(var4, var4, EPS)
                murstd8 = stat_pool.tile([4, 8], F32)
                nc.vector.tensor_copy(murstd8[:, 0:4], gmn8[:, 0:4])
                # rsqrt on DVE (bit-hack seed + Newton): no ACT table load
                y4 = stat_pool.tile([4, 4], F32)
                vi4 = y4.bitcast(I32)
                nc.vector.tensor_scalar(vi4, var4.bitcast(I32), 1, None,
                                        op0=OP.arith_shift_right)
                nc.vector.tensor_scalar(vi4, vi4, -1, 0x5F3759DF,
                                        op0=OP.mult, op1=OP.add)
                for it in range(NEWTON):
                    t4 = stat_pool.tile([4, 4], F32, name=f"t4_{it}", tag="t4")
                    nc.vector.tensor_mul(t4, y4, y4)
                    nc.vector.tensor_mul(t4, t4, var4)
                    nc.vector.tensor_scalar(t4, t4, -0.5, 1.5,
                                            op0=OP.mult, op1=OP.add)
                    nc.vector.tensor_mul(
                        murstd8[:, 4:8] if it == NEWTON - 1 else y4, y4, t4)
                cps8 = tri_ps.tile([P, 8], F32, name="cps8", tag="tri")
                nc.tensor.matmul(cps8, gmT, murstd8, start=True, stop=True)
                a_all = ab_pool.tile([P, 4], F32)
                b_all = ab_pool.tile([P, 4], F32)
                nc.vector.tensor_mul(a_all, wcols, cps8[:, 4:8])
                nc.vector.tensor_mul(b_all, cps8[:, 0:4], a_all)
                nc.vector.tensor_sub(b_all, bcols, b_all)
                return a_all, b_all

            class GN1Stats:
                """GN1 stats for sample s. Burst mode (sample 0): x staged
                in SBUF, whole-tile ops (ACT sumsq lane || DVE sum lane).
                Streamed mode: chunked x reads, one step per phase-1 nt of
                the previous sample (ACT sumsq + DVE sum per chunk)."""

                def __init__(self, s, staged=None):
                    self.s = s
                    self.ab = None
                    self.xst = list(staged) if staged else None
                    self.st8 = stat_pool.tile([P, 8], F32, name="st8g",
                                              tag="st8g")
                    self.sm8 = None
                    self.smc = None

                def _finalize(self):
                    self.ab = gn_affine8(self.st8, w1c, b1c)

                def step(self, i):
                    ct, jj = i // NCH, i % NCH
                    if jj == 0:
                        self.sm8 = stat_pool.tile([P, 8], F32, name="sm8",
                                                  tag="sm8")
                        self.smc = stat_pool.tile([P, 8], F32, name="smc",
                                                  tag="smc")
                    rows = slice(self.s * C + ct * P,
                                 self.s * C + (ct + 1) * P)
                    xc = xap_pool.tile([P, 512], BF16, name="xc", tag="xap")
                    nc.sync.dma_start(xc, x_d[rows, jj * 512:(jj + 1) * 512])
                    dmp = dump_pool.tile([P, 512], BF16)
                    nc.scalar.activation(dmp, xc, AF.Square,
                                         accum_out=self.sm8[:, jj:jj + 1])
                    nc.vector.reduce_sum(self.smc[:, jj:jj + 1], xc,
                                         axis=AX.X)
                    if jj == NCH - 1:
                        nc.vector.reduce_sum(self.st8[:, 4 + ct:5 + ct],
                                             self.sm8, axis=AX.X)
                        nc.vector.reduce_sum(self.st8[:, ct:ct + 1],
                                             self.smc, axis=AX.X)
                        if ct == CT - 1:
                            self._finalize()

                def run_burst(self):
                    # sumsq lane on ACT; sums on DVE except the last ctile,
                    # whose halves are pre-added on GPSIMD so the DVE lane
                    # is not the startup tail
                    for ct in range(CT):
                        dmp = bdump_pool.tile([P, N], BF16)
                        nc.scalar.activation(dmp, self.xst[ct], AF.Square,
                                             accum_out=self.st8[:,
                                                                4 + ct:5 + ct])
                        if ct < CT - 1:
                            nc.vector.reduce_sum(self.st8[:, ct:ct + 1],
                                                 self.xst[ct], axis=AX.X)
                    bf = bdump_pool.tile([P, N // 2], BF16)
                    nc.gpsimd.tensor_add(bf, self.xst[CT - 1][:, 0:N // 2],
                                         self.xst[CT - 1][:, N // 2:N])
                    nc.vector.reduce_sum(self.st8[:, CT - 1:CT], bf,
                                         axis=AX.X)
                    self._finalize()

            pending_stats = {}

            class Phase1:
                """GN1 apply + kv^T pass + context for one sample; split
                emission (run_nts in two parts around the previous sample's
                phase 3) hides the GN2 latency and ACT table loads."""

                def __init__(self, s, idx):
                    self.s = s
                    self.idx = idx
                    self.xn = [xn_pool.tile([P, N], BF16, name="xnt",
                                            tag="xnt") for _ in range(CT)]
                    self.ekts = []
                    self.ctx_acc = None
                    self.prev = None
                    self.nstats = None
                    self.ctx_sb = None

                def apply(self, stats, jlo, jhi):
                    a1, b1 = stats.ab
                    for j in range(jlo, jhi):
                        for ct in range(CT):
                            if stats.xst is not None:
                                xc = stats.xst[ct][:,
                                                   j * 512:(j + 1) * 512]
                            else:
                                rows = slice(self.s * C + ct * P,
                                             self.s * C + (ct + 1) * P)
                                xc = xap_pool.tile([P, 512], BF16,
                                                   name="xc", tag="xap")
                                nc.sync.dma_start(
                                    xc, x_d[rows, j * 512:(j + 1) * 512])
                            nc.vector.tensor_scalar(
                                self.xn[ct][:, j * 512:(j + 1) * 512], xc,
                                a1[:, ct:ct + 1], b1[:, ct:ct + 1],
                                op0=OP.mult, op1=OP.add)

                def attach_next_stats(self):
                    if self.idx + 1 < len(seq):
                        self.nstats = GN1Stats(seq[self.idx + 1])
                        pending_stats[self.idx + 1] = self.nstats

                def _emit_ctx(self, ekt, vt, nt):
                    for dt in range(CT):
                        nc.tensor.matmul(self.ctx_acc[dt],
                                         ekt[:, dt * P:(dt + 1) * P], vt,
                                         start=(nt == 0),
                                         stop=(nt == NT - 1))

                def run_nts(self, lo, hi):
                    if self.ctx_acc is None:
                        self.ctx_acc = [quad_ps.tile([P, C], F32,
                                                     name="ctx_acc",
                                                     tag="quad")
                                        for _ in range(CT)]
                    for nt in range(lo, hi):
                        kps = tri_ps.tile([P, 512], F32, name="kps",
                                          tag="tri")
                        for ct in range(CT):
                            nc.tensor.matmul(
                                kps, self.xn[ct][:, nt * P:(nt + 1) * P],
                                kvw_sb[:, ct * 2 * C: ct * 2 * C + 512],
                                start=(ct == 0), stop=(ct == CT - 1))
                        vps = tri_ps.tile([P, 512], F32, name="vps",
                                          tag="tri")
                        for ct in range(CT):
                            nc.tensor.matmul(
                                vps, self.xn[ct][:, nt * P:(nt + 1) * P],
                                kvw_sb[:, ct * 2 * C + 512:
                                       (ct + 1) * 2 * C],
                                start=(ct == 0), stop=(ct == CT - 1))
                        ekt = ekt_pool.tile([P, 512], BF16, name="ekt",
                                            tag="ekt")
                        nc.scalar.activation(ekt, kps, AF.Exp)  # bias cancels
                        vt = vt_pool.tile([P, 512], BF16)
                        # v-bias folded later; alternate engines per nt
                        if nt % 2 == 0:
                            nc.scalar.copy(vt, vps)
                        else:
                            nc.vector.tensor_copy(vt, vps)
                        self.ekts.append(ekt)
                        if self.nstats is not None:
                            self.nstats.step(nt)
                        if self.prev is not None:
                            self._emit_ctx(*self.prev)
                        self.prev = (ekt, vt, nt)

                def finish(self):
                    self._emit_ctx(*self.prev)
                    # R[d] = sum_n e^k via free-dim-1 riders; one PSUM
                    # accumulation group per 2KB zero region -> 4 banks.
                    r_tiles = []
                    for dt in range(CT):
                        pool_, tag_ = ((row_ps, "row") if dt == CT - 1
                                       else (tri_ps, "tri"))
                        r_tiles.append(pool_.tile([P, 1], F32,
                                                  name=f"rt{dt}", tag=tag_))
                    for nt in range(NT):
                        for dt in range(CT):
                            nc.tensor.matmul(
                                r_tiles[dt],
                                self.ekts[nt][:, dt * P:(dt + 1) * P],
                                ones_col,
                                start=(nt == 0), stop=(nt == NT - 1))
                    # ctx drain: ctx' = (ctx_raw/R)*e^kb + e^kb*vb
                    rcp2 = stat_pool.tile([P, CT], F32, name="rcp2",
                                          tag="rcp2")
                    for dt in range(CT):
                        nc.vector.reciprocal(rcp2[:, dt:dt + 1], r_tiles[dt])
                    nc.vector.tensor_mul(rcp2, rcp2, ekbc)
                    self.ctx_sb = []
                    for dt in range(CT):
                        t = ctxsb_pool.tile([P, C], BF16, name="ctx_sb",
                                            tag="ctx_sb")
                        nc.vector.scalar_tensor_tensor(
                            t, self.ctx_acc[dt], rcp2[:, dt:dt + 1],
                            vbp_sb[:, dt * C:(dt + 1) * C],
                            op0=OP.mult, op1=OP.add)
                        self.ctx_sb.append(t)

            def phase2(b, last):
                """Transpose + attention out; returns (o2sb, s2_8, q2_8)."""
                s2_8 = [stat_pool.tile([P, 8], F32, name="s2_8", tag="s2_8")
                        for _ in range(CT)]
                q2_8 = [stat_pool.tile([P, 8], F32, name="q2_8", tag="q2_8")
                        for _ in range(CT)]
                o2sb = [[None] * CT for _ in range(NCH)]
                o2ps = {}
                sps = {}
                bcs_map = {}

                def emit_rider(j, dt, ek2):
                    # S riders run at production time (small lag), so the
                    # 1/S chain completes well before the chunk drain and
                    # the PSUM-releasing muls can start immediately
                    if dt == 0:
                        sps[j] = row_ps.tile([1, 512], F32, name="sps",
                                             tag="row")
                    nc.tensor.matmul(sps[j], sclq[:, dt:dt + 1], ek2,
                                     start=(dt == 0), stop=(dt == CT - 1))
                    if dt == CT - 1:
                        rcs = small_pool.tile([1, 512], BF16, name="rcs",
                                              tag="rcs")
                        with nc.allow_low_precision(reason="bf16 1/S"):
                            nc.vector.reciprocal(rcs, sps[j][0:1, :])
                        bps = row_ps.tile([P, 512], F32, name="bps",
                                          tag="row")
                        nc.tensor.matmul(bps, ones_row, rcs,
                                         start=True, stop=True)
                        bcs = bcs_pool.tile([P, 512], F32)
                        nc.scalar.copy(bcs, bps)
                        bcs_map[j] = bcs
                        del sps[j]

                def emit_attn(j, dt, ek2):
                    for et in range(CT):
                        nc.tensor.matmul(o2ps[j][et],
                                         b.ctx_sb[dt][:, et * P:(et + 1) * P],
                                         ek2,
                                         start=(dt == 0), stop=(dt == CT - 1))
                    if dt == CT - 1:
                        bcs = bcs_map.pop(j)
                        # PSUM-releasing muls first (DVE; GPSIMD may not
                        # read PSUM), then stats: sumsq on ACT, sums via
                        # GPSIMD fold-adds + a short DVE reduce
                        for et in range(CT):
                            o2t = o2_pool.tile([P, 512], BF16, name="o2t",
                                               tag="o2t")
                            nc.vector.tensor_mul(o2t, o2ps[j][et], bcs)
                            o2sb[j][et] = o2t
                        for et in range(CT):
                            dmp = dump_pool.tile([P, 512], BF16)
                            nc.scalar.activation(
                                dmp, o2sb[j][et], AF.Square,
                                accum_out=q2_8[et][:, j:j + 1])
                            if last and j == NCH - 1:
                                # program tail: the GN2 chain is exposed, so
                                # skip the GPSIMD fold hop on its inputs
                                nc.vector.reduce_sum(s2_8[et][:, j:j + 1],
                                                     o2sb[j][et], axis=AX.X)
                                continue
                            f1 = fold_pool.tile([P, 256], BF16, name="f1",
                                                tag="f1")
                            nc.gpsimd.tensor_add(f1, o2sb[j][et][:, 0:256],
                                                 o2sb[j][et][:, 256:512])
                            f2 = fold_pool.tile([P, 128], BF16, name="f2",
                                                tag="f2")
                            nc.gpsimd.tensor_add(f2, f1[:, 0:128],
                                                 f1[:, 128:256])
                            nc.vector.reduce_sum(s2_8[et][:, j:j + 1],
                                                 f2, axis=AX.X)
                        del o2ps[j]

                pending2 = []
                rider_q = []
                for j in range(NCH):
                    o2ps[j] = [quad_ps.tile([P, 512], F32, name="o2ps",
                                            tag="quad") for _ in range(CT)]
                    for dt in range(CT):
                        tp = tri_ps.tile([P, 512], BF16, name="tp", tag="tri")
                        for bb in range(4):
                            nc.tensor.matmul(
                                tp[:, bb * P:(bb + 1) * P],
                                b.ekts[j * 4 + bb][:, dt * P:(dt + 1) * P],
                                ident, is_transpose=True)
                        ek2 = ek2_pool.tile([P, 512], BF16, name="ek2",
                                            tag="ek2")
                        # copies split DVE/ACT so neither becomes the
                        # gate; j==0 all-ACT: the previous sample's residual
                        # stts still occupy DVE at the phase-2 boundary
                        if dt in (1,) and j > 0:
                            nc.vector.tensor_copy(ek2, tp)
                        else:
                            nc.scalar.copy(ek2, tp)
                        rider_q.append((j, dt, ek2))
                        if len(rider_q) > RLAG:
                            emit_rider(*rider_q.pop(0))
                        pending2.append((j, dt, ek2))
                        if len(pending2) > PEND:
                            emit_attn(*pending2.pop(0))
                for r2 in rider_q:
                    emit_rider(*r2)
                for p2 in pending2:
                    emit_attn(*p2)
                if last:
                    # no following phase-1 window to hide the gelu table
                    # load: prefetch it in the phase-2 tail (q2_8 read pins
                    # it; no-dep dummies get hoisted to program start)
                    gdum = stat_pool.tile([P, 4], F32, name="gdum",
                                          tag="gdum")
                    nc.scalar.activation(gdum, q2_8[0][:, 0:4], gelu_f)
                return o2sb, s2_8, q2_8

            def gn2_params(s2_8, q2_8):
                st8 = stat_pool.tile([P, 8], F32)
                for et in range(CT):
                    nc.vector.reduce_sum(st8[:, et:et + 1], s2_8[et],
                                         axis=AX.X)
                    nc.vector.reduce_sum(st8[:, 4 + et:5 + et], q2_8[et],
                                         axis=AX.X)
                return gn_affine8(st8, w2c, b2c)

            def phase3(b, o2sb, ab2, row0):
                a2, b2 = ab2

                def emit_proj(j, gts):
                    for ot in range(CT):
                        # o3 lives in the tri pool: quad holds the next
                        # sample's ctx accumulators during this window
                        o3 = tri_ps.tile([P, 512], F32, name="o3", tag="tri")
                        for et in range(CT):
                            nc.tensor.matmul(
                                o3,
                                outw_sb[:, et * C + ot * P:
                                        et * C + (ot + 1) * P],
                                gts[et],
                                start=(et == 0), stop=(et == CT - 1))
                        ob_sb = outsb_pool.tile([P, 512], BF16,
                                                name="ob_sb", tag="outsb")
                        # (o3 + out_b) + xn in one DVE op
                        nc.vector.scalar_tensor_tensor(
                            ob_sb, o3, obc[:, ot:ot + 1],
                            b.xn[ot][:, j * 512:(j + 1) * 512],
                            op0=OP.add, op1=OP.add)
                        nc.sync.dma_start(
                            out_d[row0 + ot * P: row0 + (ot + 1) * P,
                                  j * 512:(j + 1) * 512], ob_sb)

                prev3 = None
                for j in range(NCH):
                    gts = []
                    for et in range(CT):
                        g = g_pool.tile([P, 512], BF16, name="g", tag="g")
                        nc.scalar.activation(g, o2sb[j][et], gelu_f,
                                             bias=b2[:, et:et + 1],
                                             scale=a2[:, et:et + 1])
                        gts.append(g)
                    if prev3 is not None:
                        emit_proj(*prev3)
                    prev3 = (j, gts)
                emit_proj(*prev3)

            # ---------------- prologue: sample 0 ----------------
            stats0 = GN1Stats(seq[0], staged=xst0)
            stats0.run_burst()
            for ct in range(CT):
                nc.sync.dma_start(
                    kvw_sb[:, ct * 2 * C:(ct + 1) * 2 * C],
                    kvw_d[ct * P:(ct + 1) * P, :])

            b = Phase1(seq[0], 0)
            b.apply(stats0, 0, NCH)
            for dt in range(CT):
                nc.sync.dma_start(vbp_sb[:, dt * C:(dt + 1) * C],
                                  vbp_d[dt * P:(dt + 1) * P, :])
            for et in range(CT):
                nc.sync.dma_start(outw_sb[:, et * C:(et + 1) * C],
                                  outw_d[et * P:(et + 1) * P, :])
            b.attach_next_stats()
            b.run_nts(0, NT)
            b.finish()

            # ---------------- steady-state sample loop ----------------
            for idx, s in enumerate(seq):
                last = idx + 1 >= len(seq)
                nxt = None
                if not last:
                    # apply chunk j0 of the NEXT sample before phase 2: its
                    # DVE work drains during phase 2's early window, so the
                    # next phase 1 can start the moment phase 2 ends
                    nxt = Phase1(seq[idx + 1], idx + 1)
                    nst = pending_stats.pop(idx + 1)
                    nxt.apply(nst, 0, 1)
                o2sb, s2_8, q2_8 = phase2(b, last)
                if nxt is not None:
                    # next sample's first nts run while the GN2 stats chain
                    # drains on DVE: the gps8 matmul would head-of-line
                    # block PE if emitted first
                    nxt.attach_next_stats()
                    nxt.run_nts(0, K_PRE)
                ab2 = gn2_params(s2_8, q2_8)
                if nxt is not None:
                    nxt.apply(nst, 1, NCH)
                phase3(b, o2sb, ab2, s * C)
                if nxt is not None:
                    nxt.run_nts(K_PRE, NT)
                    nxt.finish()
                b = nxt

    nc.compile()
    return nc


def prep_inputs(inputs):
    """Host-side prep: shard x over batch, pre-transpose/pack weights."""
    import ml_dtypes
    bf = ml_dtypes.bfloat16
    x = np.asarray(inputs["x"], dtype=np.float32)
    kv_w = np.asarray(inputs["kv_w"], dtype=np.float32)
    kv_b = np.asarray(inputs["kv_b"], dtype=np.float32)
    out_w = np.asarray(inputs["out_w"], dtype=np.float32)
    out_b = np.asarray(inputs["out_b"], dtype=np.float32)
    w1 = np.asarray(inputs["norm1_w"], dtype=np.float32)
    b1 = np.asarray(inputs["norm1_b"], dtype=np.float32)
    w2 = np.asarray(inputs["norm2_w"], dtype=np.float32)
    b2 = np.asarray(inputs["norm2_b"], dtype=np.float32)

    kvwT = np.ascontiguousarray(kv_w.T).astype(bf)       # [C, 2C]
    outwT = np.ascontiguousarray(out_w.T).astype(bf)     # [C, C]
    kb = kv_b[:C].astype(np.float64)
    vb = kv_b[C:].astype(np.float64)
    ekb = np.exp(kb)
    sclq = (ekb * np.sqrt(float(C))).astype(np.float32)
    sclq = np.ascontiguousarray(sclq.reshape(CT, P)).astype(bf)
    vbp = np.ascontiguousarray(np.outer(ekb, vb).astype(np.float32)).astype(bf)
    prm = np.stack([w1, b1, w2, b2, out_b,
                    ekb.astype(np.float32)]).reshape(6, CT, P)
    prm = np.ascontiguousarray(prm)
    ident = np.eye(P, dtype=np.float32).astype(bf)
    gmat = np.zeros((P, 4), np.float32)
    for p in range(P):
        gmat[p, p // GSIZE] = 1.0
    gmatT = np.ascontiguousarray(gmat.T)

    xs = x.reshape(B, C, N)
    in_maps = []
    for i in range(N_CORES):
        shard = np.ascontiguousarray(
            xs[i * BPC:(i + 1) * BPC].reshape(BPC * C, N)).astype(bf)
        in_maps.append({
            "x": shard, "kvwT": kvwT, "outwT": outwT, "prm": prm,
            "sclq": sclq, "vbp": vbp, "ident": ident,
            "gmat": gmat, "gmatT": gmatT,
        })
    return in_maps


_NC_CACHE = {}


def get_program(gelu: bool = True, reps: int = 1):
    key = (bool(gelu), reps)
    if key not in _NC_CACHE:
        _NC_CACHE[key] = build_program(gelu=key[0], reps=reps)
    return _NC_CACHE[key]


def run(inputs, trace: bool = False, gelu: bool = True, reps: int = 1):
    """Run on 8 cores; returns (full_output [16,512,64,64], results)."""
    nc = get_program(gelu=gelu, reps=reps)
    in_maps = prep_inputs(inputs)
    res = run_bass_kernel_spmd(nc, in_maps, core_ids=list(range(N_CORES)),
                               trace=trace)
    full = np.empty((B, C, N), np.float32)
    for i in range(N_CORES):
        full[i * BPC:(i + 1) * BPC] = np.asarray(
            res.results[i]["out"], dtype=np.float32).reshape(BPC, C, N)
    return full.reshape(B, C, H, W), res


def kernel(**inputs) -> np.ndarray:
    out, _ = run(inputs, trace=False, gelu=True)
    return out
